# revision 2
# baseline (speedup 1.0000x reference)
"""Bahdanau additive attention kernel for Trainium2 (8 NeuronCores) — v3.

Reference computation (B=32, S=4096, D=512):
    pre   = enc @ We.T + (hidden @ Wh.T + b1)[:, None, :]   # [B, S, D]
    h     = tanh(pre)
    e     = h @ w2                                          # [B, S]
    alpha = softmax(e, axis=1)
    ctx   = einsum('bs,bsd->bd', alpha, enc)                # [B, D]

All-bf16 (fp8 anywhere upstream of alpha fails the 2e-2 gate: ctx is a
weighted mean of zero-mean values, so every noise source lands at full
relative strength).

v3 over the baseline:
  - e matmuls are column-group packed: the two 512-wide e rows of an
    s-tile run CONCURRENTLY on PE column groups 0/32 (tile_position),
    halving e-matmul wall time (27 -> ~14 us of PE).
  - The two e rows land on partitions 0/32 of one PSUM bank; exp runs
    as two [1, 512] ACT ops with fused row-sum accumulation (drops the
    32 ACTIVATION_READ_ACCUMULATOR ops' overhead via fewer, direct
    accum slots).
  - Warm-up burst trimmed 24 -> 14 matmuls.
"""

import sys

if "/opt/trn_rl_repo" not in sys.path:
    sys.path.insert(0, "/opt/trn_rl_repo")

from contextlib import ExitStack

import ml_dtypes
import numpy as np

import concourse.bass as bass
import concourse.bacc as bacc
import concourse.tile as tile
from concourse import mybir
from concourse.bass_utils import run_bass_kernel_spmd

B, S, D = 32, 4096, 512
NCORES = 8
BPC = B // NCORES          # batches per core
P = 128                    # partitions
NDC = D // P               # d (contraction) chunks
NKC = D // P               # k (output channel) chunks
ST = 1024                  # s-tile size (PE/ACT/DVE granularity)
NST = S // ST              # s tiles per batch
EST = 512                  # e-row granularity
NER = S // EST             # exp rows per batch

F32 = mybir.dt.float32
BF16 = mybir.dt.bfloat16
AF = mybir.ActivationFunctionType
ALU = mybir.AluOpType


def build_bass():
    nc = bacc.Bacc()

    encT = nc.declare_dram_parameter("encT", [BPC, NST, NDC, P, ST], BF16, isOutput=False)
    weT = nc.declare_dram_parameter("weT", [NDC, P, D], BF16, isOutput=False)
    whT = nc.declare_dram_parameter("whT", [NDC, P, D], F32, isOutput=False)
    hT = nc.declare_dram_parameter("hT", [NDC, P, BPC], F32, isOutput=False)
    b1r = nc.declare_dram_parameter("b1r", [P, NKC], F32, isOutput=False)
    # w2c[p, ki, m]: column 0 holds w2[ki*128+p], columns 1..15 zero
    w2c = nc.declare_dram_parameter("w2c", [P, NKC, 16], BF16, isOutput=False)
    ones64 = nc.declare_dram_parameter("ones64", [64, 1], F32, isOutput=False)
    ctx_out = nc.declare_dram_parameter("ctx", [P, NDC, BPC], F32, isOutput=True)

    with TileKernel(nc) as tk:
        tk.build(encT, weT, whT, hT, b1r, w2c, ones64, ctx_out)
    nc.finalize()
    return nc


class TileKernel:
    def __init__(self, nc):
        self.nc = nc
        self.stack = ExitStack()
        self.tc = None

    def __enter__(self):
        self.tc = self.stack.enter_context(tile.TileContext(self.nc))
        return self

    def __exit__(self, *exc):
        return self.stack.__exit__(*exc)

    def build(self, encT, weT, whT, hT, b1r, w2c, ones64, ctx_out):
        nc, tc, ctx = self.nc, self.tc, self.stack

        singles = ctx.enter_context(tc.tile_pool(name="singles", bufs=1))
        encp = ctx.enter_context(tc.tile_pool(name="encp", bufs=2 * NST))
        htp = ctx.enter_context(tc.tile_pool(name="htp", bufs=4))
        abp = ctx.enter_context(tc.tile_pool(name="abp", bufs=8))
        junkp = ctx.enter_context(tc.tile_pool(name="junkp", bufs=3))
        smp = ctx.enter_context(tc.tile_pool(name="smp", bufs=3))
        ctxp = ctx.enter_context(tc.tile_pool(name="ctxp", bufs=2))
        dramp = ctx.enter_context(tc.tile_pool(name="dramp", bufs=2, space="DRAM"))
        prep = ctx.enter_context(tc.tile_pool(name="prep", bufs=3, space="PSUM"))
        ecp = ctx.enter_context(tc.tile_pool(name="ecp", bufs=2, space="PSUM"))

        # ---- load constants ----
        w_sb = singles.tile([P, NDC, D], BF16)
        nc.sync.dma_start(out=w_sb, in_=weT[:].rearrange("di p k -> p di k"))
        wh_sb = singles.tile([P, NDC, D], F32)
        nc.sync.dma_start(out=wh_sb, in_=whT[:].rearrange("di p k -> p di k"))
        h_sb = singles.tile([P, NDC, BPC], F32)
        nc.sync.dma_start(out=h_sb, in_=hT[:].rearrange("di p b -> p di b"))
        b1_sb = singles.tile([P, NKC], F32)
        nc.sync.dma_start(out=b1_sb, in_=b1r[:])
        w2_sb = singles.tile([P, NKC, 16], BF16)
        nc.sync.dma_start(out=w2_sb, in_=w2c[:])
        ones_sb = singles.tile([64, 1], F32)
        nc.sync.dma_start(out=ones_sb, in_=ones64[:])

        # ---- PE warm-up burst ----
        # ~5 us of dummy matmuls while the first enc tiles stream in, so the
        # HAM clock gate reaches 8/8 before real work starts.
        wpre = prep.tile([P, ST], F32, tag="pre")
        for i in range(14):
            nc.tensor.matmul(
                out=wpre[:, 0:D], lhsT=w_sb[:, 0, 0:P], rhs=w_sb[:, i % NDC, :],
                start=True, stop=True,
            )
        wjunk = singles.tile([P, 1], F32)
        nc.vector.tensor_copy(out=wjunk, in_=wpre[:, 0:1])

        # ---- c^T = Wh @ hidden^T + b1 on PE, laid out [k(part), ki, b] ----
        cps = ecp.tile([P, NKC * BPC], F32, tag="ec")
        for ki in range(NKC):
            for di in range(NDC):
                nc.tensor.matmul(
                    out=cps[:, ki * BPC:(ki + 1) * BPC],
                    lhsT=wh_sb[:, di, ki * P:(ki + 1) * P],
                    rhs=h_sb[:, di, :],
                    start=(di == 0),
                    stop=(di == NDC - 1),
                )
        c_sb = singles.tile([P, NKC, BPC], F32)
        for ki in range(NKC):
            nc.vector.tensor_scalar_add(
                out=c_sb[:, ki, :],
                in0=cps[:, ki * BPC:(ki + 1) * BPC],
                scalar1=b1_sb[:, ki:ki + 1],
            )

        # ---- main per-batch pipeline ----
        # Softmax runs UNNORMALIZED and streamed: e is bounded (|e| < ~5)
        # so exp needs no max pass; exp(e) rows are computed as soon as the
        # e matmuls land in PSUM, round-trip through DRAM (bf16) to
        # partition-broadcast, and feed the context accumulation while the
        # PE works on later s-tiles. The 1/sum(exp) normalization is
        # applied once to the final [128, NDC] context.
        for b in range(BPC):
            pd = dramp.tile([NER, EST], BF16, tag="pd")
            lparts = smp.tile([64, NST], F32, tag="lparts")
            nc.vector.memzero(lparts)
            cacc = ctxp.tile([P, NDC, NST], F32, tag="cacc")
            for st in range(NST):
                et = encp.tile([P, NDC, ST], BF16, tag="et")
                nc.sync.dma_start(out=et, in_=encT[:][b, st].rearrange("di p s -> p di s"))

                ht = htp.tile([P, NKC, ST], BF16, tag="ht")
                for ki in range(NKC):
                    pre_ps = prep.tile([P, ST], F32, tag="pre")
                    for half in range(ST // EST):
                        sl = slice(half * EST, (half + 1) * EST)
                        for di in range(NDC):
                            nc.tensor.matmul(
                                out=pre_ps[:, sl],
                                lhsT=w_sb[:, di, ki * P:(ki + 1) * P],
                                rhs=et[:, di, sl],
                                start=(di == 0),
                                stop=(di == NDC - 1),
                            )
                    # h^T = tanh(pre^T + c), one [128, ST] ACT op per ki
                    nc.scalar.activation(
                        out=ht[:, ki, :],
                        in_=pre_ps,
                        func=AF.Tanh,
                        bias=c_sb[:, ki, b:b + 1],
                        scale=1.0,
                    )
                # e rows for both halves, packed on PE column groups 0/32:
                # the two accumulation chains run concurrently (separate
                # XBUS streams, separate PSUM banks + partition ranges).
                e_ps0 = ecp.tile([16, EST], F32, tag="ec")
                e_ps1 = ecp.tile([48, EST], F32, tag="ec")
                e_pss = [e_ps0, e_ps1]
                for ki in range(NKC):
                    for g in range(ST // EST):
                        sl = slice(g * EST, (g + 1) * EST)
                        nc.tensor.matmul(
                            out=e_pss[g][32 * g:32 * g + 16, :],
                            lhsT=w2_sb[:, ki, :],
                            rhs=ht[:, ki, sl],
                            start=(ki == 0),
                            stop=(ki == NKC - 1),
                            tile_position=(0, 32 * g),
                        )
                # p = exp(e) per row with the row-sum fused; bf16 rows go
                # out through DRAM so DMA can replicate across partitions.
                p_rows = smp.tile([48, EST], BF16, tag="prow")
                for g in range(ST // EST):
                    nc.scalar.activation(
                        out=p_rows[32 * g:32 * g + 1, :],
                        in_=e_pss[g][32 * g:32 * g + 1, :],
                        func=AF.Exp, bias=0.0, scale=1.0,
                        accum_out=lparts[32 * g:32 * g + 1, st:st + 1],
                    )
                for g in range(ST // EST):
                    r = st * (ST // EST) + g
                    nc.gpsimd.dma_start(
                        out=pd[r:r + 1, :], in_=p_rows[32 * g:32 * g + 1, :])
                # broadcast the two p rows of this s-tile in one DMA (rows
                # are contiguous in DRAM) and accumulate p * enc on DVE.
                ab = abp.tile([P, ST], BF16, tag="ab")
                rows = pd[st * (ST // EST):(st + 1) * (ST // EST), :]
                nc.gpsimd.dma_start(
                    out=ab,
                    in_=bass.AP(
                        tensor=rows.tensor,
                        offset=rows.offset,
                        ap=[[0, P], [1, ST]],
                    ),
                )
                for di in range(NDC):
                    junk = junkp.tile([P, ST], BF16, tag="junk")
                    nc.vector.scalar_tensor_tensor(
                        out=junk,
                        in0=et[:, di, :],
                        scalar=1.0,
                        in1=ab,
                        op0=ALU.mult,
                        op1=ALU.mult,
                        accum_out=cacc[:, di, st:st + 1],
                    )

            # ---- finalize: ctx = (sum_s p*enc) / sum_s p ----
            lsum2 = smp.tile([64, 1], F32, tag="lsum2")
            nc.vector.reduce_sum(out=lsum2, in_=lparts, axis=mybir.AxisListType.X)
            # cross-partition sum of rows 0/32 (rest are zero) on the PE
            ls_ps = ecp.tile([1, 1], F32, tag="ec")
            nc.tensor.matmul(out=ls_ps, lhsT=lsum2, rhs=ones_sb, start=True, stop=True)
            rinv1 = smp.tile([1, 1], F32, tag="rinv1")
            nc.vector.reciprocal(out=rinv1, in_=ls_ps)
            rinvb = smp.tile([P, 1], F32, tag="rinvb")
            nc.gpsimd.partition_broadcast(out_ap=rinvb, in_ap=rinv1)
            ctx_acc = ctxp.tile([P, NDC], F32, tag="ctx")
            nc.vector.reduce_sum(out=ctx_acc, in_=cacc, axis=mybir.AxisListType.X)
            nc.vector.tensor_scalar_mul(out=ctx_acc, in0=ctx_acc, scalar1=rinvb)
            nc.gpsimd.dma_start(out=ctx_out[:][:, :, b], in_=ctx_acc)


_NC_CACHE = None


def _get_nc():
    global _NC_CACHE
    if _NC_CACHE is None:
        _NC_CACHE = build_bass()
    return _NC_CACHE


def _prep_core_inputs(hidden_state, encoder_outputs, W1, b1, w2, core):
    bf16 = ml_dtypes.bfloat16
    b0 = core * BPC
    enc = encoder_outputs[b0:b0 + BPC]                      # [BPC, S, D] f32
    # [b, d, s] -> [b, di, p, s] -> [b, st, di, p, s]
    e = enc.transpose(0, 2, 1).reshape(BPC, NDC, P, NST, ST)
    e = np.ascontiguousarray(e.transpose(0, 3, 1, 2, 4)).astype(bf16)
    w2cv = np.zeros((P, NKC, 16), dtype=np.float32)
    w2cv[:, :, 0] = w2.reshape(NKC, P).T
    return {
        "encT": e,
        "weT": np.ascontiguousarray(W1[:, :D].T.reshape(NDC, P, D)).astype(bf16),
        "whT": np.ascontiguousarray(W1[:, D:].T.reshape(NDC, P, D)),
        "hT": np.ascontiguousarray(hidden_state[b0:b0 + BPC].T.reshape(NDC, P, BPC)),
        "b1r": np.ascontiguousarray(b1.reshape(NKC, P).T),
        "w2c": w2cv.astype(bf16),
        "ones64": np.ones((64, 1), dtype=np.float32),
    }


def kernel(hidden_state, encoder_outputs, W1, b1, w2, _trace=False, _trace_kwargs=None):
    hidden_state = np.asarray(hidden_state, dtype=np.float32)
    encoder_outputs = np.asarray(encoder_outputs, dtype=np.float32)
    W1 = np.asarray(W1, dtype=np.float32)
    b1 = np.asarray(b1, dtype=np.float32)
    w2 = np.asarray(w2, dtype=np.float32)

    nc = _get_nc()
    in_maps = [
        _prep_core_inputs(hidden_state, encoder_outputs, W1, b1, w2, c)
        for c in range(NCORES)
    ]
    res = run_bass_kernel_spmd(
        nc, in_maps, list(range(NCORES)), trace=_trace,
        **(_trace_kwargs or {}),
    )
    out = np.empty((B, D), dtype=np.float32)
    for c in range(NCORES):
        r = res.results[c]["ctx"]                          # [p, di, b]
        out[c * BPC:(c + 1) * BPC] = r.transpose(2, 1, 0).reshape(BPC, D)
    if _trace:
        return out, res
    return out


# revision 3
# speedup vs baseline: 1.0938x; 1.0938x over previous
"""Bahdanau additive attention kernel for Trainium2 (8 NeuronCores) — v3.

Reference computation (B=32, S=4096, D=512):
    pre   = enc @ We.T + (hidden @ Wh.T + b1)[:, None, :]   # [B, S, D]
    h     = tanh(pre)
    e     = h @ w2                                          # [B, S]
    alpha = softmax(e, axis=1)
    ctx   = einsum('bs,bsd->bd', alpha, enc)                # [B, D]

All-bf16 (fp8 anywhere upstream of alpha fails the 2e-2 gate: ctx is a
weighted mean of zero-mean values, so every noise source lands at full
relative strength).

v3 over the baseline:
  - e matmuls are column-group packed: the two 512-wide e rows of an
    s-tile run CONCURRENTLY on PE column groups 0/32 (tile_position),
    halving e-matmul wall time (27 -> ~14 us of PE).
  - The two e rows land on partitions 0/32 of one PSUM bank; exp runs
    as two [1, 512] ACT ops with fused row-sum accumulation (drops the
    32 ACTIVATION_READ_ACCUMULATOR ops' overhead via fewer, direct
    accum slots).
  - Warm-up burst trimmed 24 -> 14 matmuls.
"""

import sys

if "/opt/trn_rl_repo" not in sys.path:
    sys.path.insert(0, "/opt/trn_rl_repo")

from contextlib import ExitStack

import ml_dtypes
import numpy as np

import concourse.bass as bass
import concourse.bacc as bacc
import concourse.tile as tile
from concourse import mybir
from concourse.bass_utils import run_bass_kernel_spmd

B, S, D = 32, 4096, 512
NCORES = 8
BPC = B // NCORES          # batches per core
P = 128                    # partitions
NDC = D // P               # d (contraction) chunks
NKC = D // P               # k (output channel) chunks
ST = 1024                  # s-tile size (PE/ACT/DVE granularity)
NST = S // ST              # s tiles per batch
EST = 512                  # e-row granularity
NER = S // EST             # exp rows per batch

F32 = mybir.dt.float32
BF16 = mybir.dt.bfloat16
AF = mybir.ActivationFunctionType
ALU = mybir.AluOpType


def build_bass():
    nc = bacc.Bacc()

    encT = nc.declare_dram_parameter("encT", [BPC, NST, NDC, P, ST], BF16, isOutput=False)
    weT = nc.declare_dram_parameter("weT", [NDC, P, D], BF16, isOutput=False)
    whT = nc.declare_dram_parameter("whT", [NDC, P, D], F32, isOutput=False)
    hT = nc.declare_dram_parameter("hT", [NDC, P, BPC], F32, isOutput=False)
    b1r = nc.declare_dram_parameter("b1r", [P, NKC], F32, isOutput=False)
    # w2c[p, ki, m]: column 0 holds w2[ki*128+p], columns 1..15 zero
    w2c = nc.declare_dram_parameter("w2c", [P, NKC, 16], BF16, isOutput=False)
    ctx_out = nc.declare_dram_parameter("ctx", [P, NDC, BPC], F32, isOutput=True)

    with TileKernel(nc) as tk:
        tk.build(encT, weT, whT, hT, b1r, w2c, ctx_out)
    nc.finalize()
    return nc


class TileKernel:
    def __init__(self, nc):
        self.nc = nc
        self.stack = ExitStack()
        self.tc = None

    def __enter__(self):
        self.tc = self.stack.enter_context(tile.TileContext(self.nc))
        return self

    def __exit__(self, *exc):
        return self.stack.__exit__(*exc)

    def build(self, encT, weT, whT, hT, b1r, w2c, ctx_out):
        nc, tc, ctx = self.nc, self.tc, self.stack

        singles = ctx.enter_context(tc.tile_pool(name="singles", bufs=1))
        encp = ctx.enter_context(tc.tile_pool(name="encp", bufs=2 * NST))
        htp = ctx.enter_context(tc.tile_pool(name="htp", bufs=4))
        abp = ctx.enter_context(tc.tile_pool(name="abp", bufs=8))
        junkp = ctx.enter_context(tc.tile_pool(name="junkp", bufs=3))
        smp = ctx.enter_context(tc.tile_pool(name="smp", bufs=3))
        ctxp = ctx.enter_context(tc.tile_pool(name="ctxp", bufs=2))
        dramp = ctx.enter_context(tc.tile_pool(name="dramp", bufs=2, space="DRAM"))
        prep = ctx.enter_context(tc.tile_pool(name="prep", bufs=3, space="PSUM"))
        ecp = ctx.enter_context(tc.tile_pool(name="ecp", bufs=2, space="PSUM"))

        # ---- load constants ----
        # w_sb rides the sync queue ahead of the enc tiles; the other
        # constants go via the gpsimd queue so they don't delay enc tile 0.
        w_sb = singles.tile([P, NDC, D], BF16)
        nc.sync.dma_start(out=w_sb, in_=weT[:].rearrange("di p k -> p di k"))
        wh_sb = singles.tile([P, NDC, D], F32)
        nc.gpsimd.dma_start(out=wh_sb, in_=whT[:].rearrange("di p k -> p di k"))
        h_sb = singles.tile([P, NDC, BPC], F32)
        nc.gpsimd.dma_start(out=h_sb, in_=hT[:].rearrange("di p b -> p di b"))
        b1_sb = singles.tile([P, NKC], F32)
        nc.gpsimd.dma_start(out=b1_sb, in_=b1r[:])
        w2_sb = singles.tile([P, NKC, 16], BF16)
        nc.gpsimd.dma_start(out=w2_sb, in_=w2c[:])

        # ---- PE warm-up burst ----
        # ~6 us of dummy matmuls on a zeroed tile (no DMA dependency) so the
        # HAM clock gate reaches 8/8 before real work starts.
        wz = singles.tile([P, D], BF16)
        nc.vector.memzero(wz)
        wpre = prep.tile([P, ST], F32, tag="pre")
        for i in range(14):
            nc.tensor.matmul(
                out=wpre[:, 0:D], lhsT=wz[:, 0:P], rhs=wz,
                start=True, stop=True,
            )
        wjunk = singles.tile([P, 1], F32)
        nc.vector.tensor_copy(out=wjunk, in_=wpre[:, 0:1])

        # ---- c^T = Wh @ hidden^T + b1 on PE, laid out [k(part), ki, b] ----
        cps = ecp.tile([P, NKC * BPC], F32, tag="ec")
        for ki in range(NKC):
            for di in range(NDC):
                nc.tensor.matmul(
                    out=cps[:, ki * BPC:(ki + 1) * BPC],
                    lhsT=wh_sb[:, di, ki * P:(ki + 1) * P],
                    rhs=h_sb[:, di, :],
                    start=(di == 0),
                    stop=(di == NDC - 1),
                )
        c_sb = singles.tile([P, NKC, BPC], F32)
        for ki in range(NKC):
            nc.vector.tensor_scalar_add(
                out=c_sb[:, ki, :],
                in0=cps[:, ki * BPC:(ki + 1) * BPC],
                scalar1=b1_sb[:, ki:ki + 1],
            )

        # ---- main per-batch pipeline ----
        # Softmax runs UNNORMALIZED and streamed: e is bounded (|e| < ~5)
        # so exp needs no max pass; exp(e) rows are computed as soon as the
        # e matmuls land in PSUM, round-trip through DRAM (bf16) to
        # partition-broadcast, and feed the context accumulation while the
        # PE works on later s-tiles. The 1/sum(exp) normalization is
        # applied once to the final [128, NDC] context.
        for b in range(BPC):
            pd = dramp.tile([NER, EST], BF16, tag="pd")
            lparts = smp.tile([64, NST], F32, tag="lparts")
            nc.vector.memzero(lparts)
            cacc = ctxp.tile([P, NDC, NST], F32, tag="cacc")
            for st in range(NST):
                et = encp.tile([P, NDC, ST], BF16, tag="et")
                nc.sync.dma_start(out=et, in_=encT[:][b, st].rearrange("di p s -> p di s"))

                ht = htp.tile([P, NKC, ST], BF16, tag="ht")
                for ki in range(NKC):
                    pre_ps = prep.tile([P, ST], F32, tag="pre")
                    for half in range(ST // EST):
                        sl = slice(half * EST, (half + 1) * EST)
                        for di in range(NDC):
                            nc.tensor.matmul(
                                out=pre_ps[:, sl],
                                lhsT=w_sb[:, di, ki * P:(ki + 1) * P],
                                rhs=et[:, di, sl],
                                start=(di == 0),
                                stop=(di == NDC - 1),
                            )
                    # h^T = tanh(pre^T + c), one [128, ST] ACT op per ki
                    nc.scalar.activation(
                        out=ht[:, ki, :],
                        in_=pre_ps,
                        func=AF.Tanh,
                        bias=c_sb[:, ki, b:b + 1],
                        scale=1.0,
                    )
                # e rows for both halves, packed on PE column groups 0/32:
                # the two accumulation chains run concurrently (separate
                # XBUS streams, separate PSUM banks + partition ranges).
                e_ps0 = ecp.tile([16, EST], F32, tag="ec")
                e_ps1 = ecp.tile([48, EST], F32, tag="ec")
                e_pss = [e_ps0, e_ps1]
                for ki in range(NKC):
                    for g in range(ST // EST):
                        sl = slice(g * EST, (g + 1) * EST)
                        nc.tensor.matmul(
                            out=e_pss[g][32 * g:32 * g + 16, :],
                            lhsT=w2_sb[:, ki, :],
                            rhs=ht[:, ki, sl],
                            start=(ki == 0),
                            stop=(ki == NKC - 1),
                            tile_position=(0, 32 * g),
                        )
                # p = exp(e) per row with the row-sum fused; bf16 rows go
                # out through DRAM so DMA can replicate across partitions.
                p_rows = smp.tile([48, EST], BF16, tag="prow")
                for g in range(ST // EST):
                    nc.scalar.activation(
                        out=p_rows[32 * g:32 * g + 1, :],
                        in_=e_pss[g][32 * g:32 * g + 1, :],
                        func=AF.Exp, bias=0.0, scale=1.0,
                        accum_out=lparts[32 * g:32 * g + 1, st:st + 1],
                    )
                for g in range(ST // EST):
                    r = st * (ST // EST) + g
                    nc.gpsimd.dma_start(
                        out=pd[r:r + 1, :], in_=p_rows[32 * g:32 * g + 1, :])
                # broadcast the two p rows of this s-tile across partitions,
                # split into two half DMAs on separate queues so the
                # replication bandwidth doubles.
                ab = abp.tile([P, ST], BF16, tag="ab")
                for g, issuer in ((0, nc.gpsimd), (1, nc.sync)):
                    r = st * (ST // EST) + g
                    rows = pd[r:r + 1, :]
                    issuer.dma_start(
                        out=ab[:, g * EST:(g + 1) * EST],
                        in_=bass.AP(
                            tensor=rows.tensor,
                            offset=rows.offset,
                            ap=[[0, P], [1, EST]],
                        ),
                    )
                for di in range(NDC):
                    junk = junkp.tile([P, ST], BF16, tag="junk")
                    nc.vector.scalar_tensor_tensor(
                        out=junk,
                        in0=et[:, di, :],
                        scalar=1.0,
                        in1=ab,
                        op0=ALU.mult,
                        op1=ALU.mult,
                        accum_out=cacc[:, di, st:st + 1],
                    )

            # ---- finalize: ctx = (sum_s p*enc) / sum_s p ----
            lsum2 = smp.tile([64, 1], F32, tag="lsum2")
            nc.vector.reduce_sum(out=lsum2, in_=lparts, axis=mybir.AxisListType.X)
            # gather the two per-row partials onto one partition via tiny
            # SBUF->SBUF DMAs (keeps the PE queue out of the softmax path)
            lpair = smp.tile([1, 2], F32, tag="lpair")
            nc.gpsimd.dma_start(out=lpair[:, 0:1], in_=lsum2[0:1, :])
            nc.gpsimd.dma_start(out=lpair[:, 1:2], in_=lsum2[32:33, :])
            lsum = smp.tile([1, 1], F32, tag="lsum")
            nc.vector.reduce_sum(out=lsum, in_=lpair, axis=mybir.AxisListType.X)
            rinv1 = smp.tile([1, 1], F32, tag="rinv1")
            nc.vector.reciprocal(out=rinv1, in_=lsum)
            rinvb = smp.tile([P, 1], F32, tag="rinvb")
            nc.gpsimd.partition_broadcast(out_ap=rinvb, in_ap=rinv1)
            ctx_acc = ctxp.tile([P, NDC], F32, tag="ctx")
            nc.vector.reduce_sum(out=ctx_acc, in_=cacc, axis=mybir.AxisListType.X)
            nc.vector.tensor_scalar_mul(out=ctx_acc, in0=ctx_acc, scalar1=rinvb)
            nc.gpsimd.dma_start(out=ctx_out[:][:, :, b], in_=ctx_acc)


_NC_CACHE = None


def _get_nc():
    global _NC_CACHE
    if _NC_CACHE is None:
        _NC_CACHE = build_bass()
    return _NC_CACHE


def _prep_core_inputs(hidden_state, encoder_outputs, W1, b1, w2, core):
    bf16 = ml_dtypes.bfloat16
    b0 = core * BPC
    enc = encoder_outputs[b0:b0 + BPC]                      # [BPC, S, D] f32
    # [b, d, s] -> [b, di, p, s] -> [b, st, di, p, s]
    e = enc.transpose(0, 2, 1).reshape(BPC, NDC, P, NST, ST)
    e = np.ascontiguousarray(e.transpose(0, 3, 1, 2, 4)).astype(bf16)
    w2cv = np.zeros((P, NKC, 16), dtype=np.float32)
    w2cv[:, :, 0] = w2.reshape(NKC, P).T
    return {
        "encT": e,
        "weT": np.ascontiguousarray(W1[:, :D].T.reshape(NDC, P, D)).astype(bf16),
        "whT": np.ascontiguousarray(W1[:, D:].T.reshape(NDC, P, D)),
        "hT": np.ascontiguousarray(hidden_state[b0:b0 + BPC].T.reshape(NDC, P, BPC)),
        "b1r": np.ascontiguousarray(b1.reshape(NKC, P).T),
        "w2c": w2cv.astype(bf16),
    }


def kernel(hidden_state, encoder_outputs, W1, b1, w2, _trace=False, _trace_kwargs=None):
    hidden_state = np.asarray(hidden_state, dtype=np.float32)
    encoder_outputs = np.asarray(encoder_outputs, dtype=np.float32)
    W1 = np.asarray(W1, dtype=np.float32)
    b1 = np.asarray(b1, dtype=np.float32)
    w2 = np.asarray(w2, dtype=np.float32)

    nc = _get_nc()
    in_maps = [
        _prep_core_inputs(hidden_state, encoder_outputs, W1, b1, w2, c)
        for c in range(NCORES)
    ]
    res = run_bass_kernel_spmd(
        nc, in_maps, list(range(NCORES)), trace=_trace,
        **(_trace_kwargs or {}),
    )
    out = np.empty((B, D), dtype=np.float32)
    for c in range(NCORES):
        r = res.results[c]["ctx"]                          # [p, di, b]
        out[c * BPC:(c + 1) * BPC] = r.transpose(2, 1, 0).reshape(BPC, D)
    if _trace:
        return out, res
    return out
